# revision 13
# baseline (speedup 1.0000x reference)
"""Trainium2 Bass kernel for nn_Decoder (2-layer GRU decoder, weight-tied vocab projection).

Strategy (8 NeuronCores, SPMD):
  - Tensor-parallel recurrence: each core computes a 128-row slice of every GRU
    gate (H=1024 -> 8 x 128). Per superstep, one fused AllGather exchanges the
    new h0/h1 slices (bf16) across cores.
  - Gate preactivations use the TRANSPOSED matmul form: h K-tiles [128,16] are
    the stationary lhsT, weight blocks [128,384] are the moving rhs, so each
    layer-matrix needs only 8 matmul instructions (vs 24 with W stationary).
    Gates are evaluated batch-major [16,384]; one PE transpose per layer per
    step returns the h-major [128,16] slice for the AllGather.
  - Vocab-tied projection: embedding^T is sharded 4000 cols/core; the MLP is
    computed replicated (cheap) and logits are vocab-sharded.
Output: (B=16, S=128, V=32000) fp32, assembled host-side.
"""
import os
import numpy as np
import ml_dtypes

V, E, H, L = 32000, 512, 1024, 2
B, S = 16, 128
N = 8                  # cores
HS = H // N            # 128 rows of h per core
VS = V // N            # 4000 vocab cols per core
TB = B * S             # 2048 (t,b) rows
G3 = 3 * HS            # 384 gate rows per core
STEPS = int(os.environ.get("K_STEPS", str(S)))  # reduced for smoke testing

_cache = {}


def _build(USE_BIAS=True):
    import concourse.bass as bass
    import concourse.bacc as bacc
    import concourse.mybir as mybir
    import concourse.tile as tile
    from concourse.masks import make_identity

    fp32 = mybir.dt.float32
    bf16 = mybir.dt.bfloat16
    i32 = mybir.dt.int32
    T = STEPS
    NT = (T * B) // 128   # number of 128-row (t,b) tiles

    nc = bacc.Bacc("TRN2", num_devices=N, target_bir_lowering=False)

    # ---- DRAM I/O ----
    emb = nc.dram_tensor("emb", [V, E], fp32, kind="ExternalInput")
    idx = nc.dram_tensor("idx", [NT, 128], i32, kind="ExternalInput")
    h0f = nc.dram_tensor("h0f", [H, B], bf16, kind="ExternalInput")
    h1f = nc.dram_tensor("h1f", [H, B], bf16, kind="ExternalInput")
    h0o = nc.dram_tensor("h0o", [B, HS], fp32, kind="ExternalInput")
    h1o = nc.dram_tensor("h1o", [B, HS], fp32, kind="ExternalInput")
    wih0 = nc.dram_tensor("wih0", [E, G3], bf16, kind="ExternalInput")
    whh0 = nc.dram_tensor("whh0", [H, G3], bf16, kind="ExternalInput")
    wih1 = nc.dram_tensor("wih1", [H, G3], bf16, kind="ExternalInput")
    whh1 = nc.dram_tensor("whh1", [H, G3], bf16, kind="ExternalInput")
    ba0 = nc.dram_tensor("ba0", [1, G3], bf16, kind="ExternalInput")
    bb0p = nc.dram_tensor("bb0p", [1, G3], bf16, kind="ExternalInput")
    ba1 = nc.dram_tensor("ba1", [1, G3], bf16, kind="ExternalInput")
    bb1p = nc.dram_tensor("bb1p", [1, G3], bf16, kind="ExternalInput")
    w1t = nc.dram_tensor("w1t", [H, H], bf16, kind="ExternalInput")
    b1c = nc.dram_tensor("b1c", [128, 8], fp32, kind="ExternalInput")
    w2t = nc.dram_tensor("w2t", [H, E], bf16, kind="ExternalInput")
    b2c = nc.dram_tensor("b2c", [128, 4], fp32, kind="ExternalInput")
    embts = nc.dram_tensor("embts", [E, VS], bf16, kind="ExternalInput")
    bgen = nc.dram_tensor("bgen", [1, VS], bf16, kind="ExternalInput")
    out = nc.dram_tensor("out", [T * B, VS], fp32, kind="ExternalOutput")

    with tile.TileContext(nc) as tc:
        with (
            tc.tile_pool(name="wp", bufs=1) as wp,
            tc.tile_pool(name="state", bufs=1) as st,
            tc.tile_pool(name="dram", bufs=4, space="DRAM") as dr,
        ):
            # ---- resident weights ----
            def load3d(name, src, kdim, cols):
                # src (kdim*128, cols) -> sbuf (128, kdim*cols), k-major blocks
                tl = wp.tile([128, kdim * cols], bf16, tag=name, name=name)
                nc.sync.dma_start(
                    tl[:].rearrange("p (k c) -> p k c", k=kdim),
                    src[:].rearrange("(k p) c -> p k c", p=128))
                return tl

            wih0_sb = load3d("wih0_sb", wih0, 4, G3)
            whh0_sb = load3d("whh0_sb", whh0, 8, G3)
            wih1_sb = load3d("wih1_sb", wih1, 8, G3)
            whh1_sb = load3d("whh1_sb", whh1, 8, G3)
            w1t_sb = load3d("w1t_sb", w1t, 8, H)
            w2t_sb = load3d("w2t_sb", w2t, 8, E)
            embts_sb = load3d("embts_sb", embts, 4, VS)
            ba0_sb = wp.tile([1, G3], bf16, tag="ba0s", name="ba0s")
            nc.sync.dma_start(ba0_sb[:], ba0[:])
            bb0p_sb = wp.tile([1, G3], bf16, tag="bb0ps", name="bb0ps")
            nc.sync.dma_start(bb0p_sb[:], bb0p[:])
            ba1_sb = wp.tile([1, G3], bf16, tag="ba1s", name="ba1s")
            nc.sync.dma_start(ba1_sb[:], ba1[:])
            bb1p_sb = wp.tile([1, G3], bf16, tag="bb1ps", name="bb1ps")
            nc.sync.dma_start(bb1p_sb[:], bb1p[:])
            b1_sb = wp.tile([128, 8], fp32, tag="b1s", name="b1s")
            nc.sync.dma_start(b1_sb[:], b1c[:])
            b2_sb = wp.tile([128, 4], fp32, tag="b2s", name="b2s")
            nc.sync.dma_start(b2_sb[:], b2c[:])
            bgen_sb = wp.tile([128, VS], bf16, tag="bgens", name="bgens")
            nc.sync.dma_start(bgen_sb[0:1, :], bgen[:])
            ones_sb = wp.tile([128, 512], bf16, tag="ones", name="ones")
            nc.vector.memset(ones_sb[:], 1.0)
            ident = wp.tile([128, 128], fp32, tag="ident", name="ident")
            make_identity(nc, ident[:])

            # ---- state ----
            XT = st.tile([128, 4 * (T * B)], bf16, tag="XT", name="XT")        # x^T, 4 E-tiles
            # l0 x-gates live in DRAM ([tb, 384] rows); the DVE is lane-aligned
            # (SBUF operands must share base partition), so each step's [16,384]
            # block is DMA-prefetched into a partition-0 ring two steps ahead.
            GI_d = dr.tile([T * B, G3], bf16, tag="gid", name="gid", bufs=1)
            gis_ring = st.tile([16, 4 * G3], bf16, tag="gisr", name="gisr")
            ring0 = st.tile([128, 4 * 128], bf16, tag="ring0", name="ring0")   # h0 ring, 4 slots x (8k x 16)
            H1T = st.tile([128, (T + 1) * 128], bf16, tag="H1T", name="H1T")   # slot-major
            h_own0 = [st.tile([16, HS], fp32, tag=f"ho0_{p}", name=f"ho0_{p}") for p in range(2)]
            h_own1 = [st.tile([16, HS], fp32, tag=f"ho1_{p}", name=f"ho1_{p}") for p in range(2)]

            # init h state
            nc.sync.dma_start(
                ring0[:, 0:128].rearrange("p (k c) -> p k c", k=8),
                h0f[:].rearrange("(k p) c -> p k c", p=128))
            nc.sync.dma_start(
                H1T[:, 0:128].rearrange("p (k c) -> p k c", k=8),
                h1f[:].rearrange("(k p) c -> p k c", p=128))
            nc.sync.dma_start(h_own0[0][:], h0o[:])
            nc.sync.dma_start(h_own1[0][:], h1o[:])

            # ---- P1: gather + transpose x ----
            with (
                tc.tile_pool(name="gp", bufs=2) as gp,
                tc.tile_pool(name="gps", bufs=2, space="PSUM") as gpp,
            ):
                idx_sb = gp.tile([128, NT], i32, tag="idx", name="idx")
                nc.sync.dma_start(idx_sb[:], idx[:].rearrange("a b -> b a"))
                for i in range(NT):
                    xg = gp.tile([128, E], fp32, tag="xg", name=f"xg{i}", bufs=2)
                    nc.gpsimd.indirect_dma_start(
                        out=xg[:], out_offset=None, in_=emb[:],
                        in_offset=bass.IndirectOffsetOnAxis(ap=idx_sb[:, i:i + 1], axis=0))
                    for e in range(4):
                        pt = gpp.tile([128, 128], fp32, tag="pt", name=f"pt{i}_{e}")
                        nc.tensor.transpose(pt[:], xg[:, 128 * e:128 * (e + 1)], ident[:])
                        nc.vector.tensor_copy(
                            XT[:, (T * B) * e + 128 * i: (T * B) * e + 128 * (i + 1)], pt[:])

            # ---- P2a: bulk input-gate precompute for layer 0 -> DRAM [tb, 384] ----
            with (
                tc.tile_pool(name="gi_sb", bufs=2) as gsb,
                tc.tile_pool(name="gi_ps", bufs=2, space="PSUM") as gip,
            ):
                for g in range(NT):
                    pg = gip.tile([128, G3], fp32, tag="pg", name=f"pg{g}", bufs=2)
                    for e in range(4):
                        nc.tensor.matmul(
                            pg[:], XT[:, (T * B) * e + 128 * g:(T * B) * e + 128 * (g + 1)],
                            wih0_sb[:, G3 * e:G3 * (e + 1)],
                            start=(e == 0), stop=(not USE_BIAS and e == 3))
                    if USE_BIAS:
                        nc.tensor.matmul(
                            pg[:], ones_sb[0:1, 0:128], ba0_sb[0:1, :],
                            start=False, stop=True)
                    stg = gsb.tile([128, G3], bf16, tag="stg", name=f"stg{g}", bufs=2)
                    nc.vector.tensor_copy(stg[:], pg[:])
                    nc.scalar.dma_start(GI_d[128 * g:128 * (g + 1), :], stg[:])

            # ---- P2: recurrence supersteps ----
            def gru_gates(psA_rz, psA_n, sb_rz, sb_n, hprev, hnew, tag, t):
                # psA: h-recurrent preact in PSUM (r,z | n-h-part);
                # sb: input preact in SBUF (r,z | n-x-part)
                # rz = sigmoid(psA_rz + sb_rz); n = tanh(sb_n + r * psA_n)
                # hnew = n + z * (hprev - n)
                trz = st.tile([16, 256], fp32, tag=f"trz{tag}", name=f"trz{tag}_{t}", bufs=2)
                nc.vector.tensor_tensor(out=trz[:], in0=psA_rz, in1=sb_rz,
                                        op=mybir.AluOpType.add)
                rz = st.tile([16, 256], fp32, tag=f"rz{tag}", name=f"rz{tag}_{t}", bufs=2)
                nc.scalar.activation(rz[:], trz[:], mybir.ActivationFunctionType.Sigmoid)
                t1 = st.tile([16, HS], fp32, tag=f"t1{tag}", name=f"t1{tag}_{t}", bufs=2)
                nc.vector.tensor_tensor(out=t1[:], in0=psA_n, in1=rz[:, 0:128],
                                        op=mybir.AluOpType.mult)
                nc.vector.tensor_tensor(out=t1[:], in0=t1[:], in1=sb_n,
                                        op=mybir.AluOpType.add)
                nsb = st.tile([16, HS], fp32, tag=f"n{tag}", name=f"n{tag}_{t}", bufs=2)
                nc.scalar.activation(nsb[:], t1[:], mybir.ActivationFunctionType.Tanh)
                a = st.tile([16, HS], fp32, tag=f"a{tag}", name=f"a{tag}_{t}", bufs=2)
                nc.vector.tensor_tensor(out=a[:], in0=hprev[:], in1=nsb[:],
                                        op=mybir.AluOpType.subtract)
                nc.vector.tensor_tensor(out=a[:], in0=a[:], in1=rz[:, 128:256],
                                        op=mybir.AluOpType.mult)
                nc.vector.tensor_tensor(out=hnew[:], in0=a[:], in1=nsb[:],
                                        op=mybir.AluOpType.add)

            psp_cm = tc.tile_pool(name="ps", bufs=1, space="PSUM")
            psp = psp_cm.__enter__()
            mp_cm = tc.tile_pool(name="mp", bufs=1)
            mp = mp_cm.__enter__()
            mpp_cm = tc.tile_pool(name="mps", bufs=1, space="PSUM")
            mpp = mpp_cm.__enter__()

            GC = 128
            NG = (T * B) // GC
            H1T3 = H1T[:].rearrange("p (t c) -> p t c", c=128)

            cc_srcs = {t: st.tile([128, 32], bf16, tag="cc_src", name=f"cc_src{t}", bufs=4)
                       for t in range(1, T + 5)}
            for tt in (1, 2):
                nc.vector.memset(cc_srcs[tt][:, 16:32], 0.0)
            for tt in (T + 1, T + 2):
                nc.vector.memset(cc_srcs[tt][:, 0:16], 0.0)

            p5_state = {}

            def p5_items(g):
                # returns list of emission closures for tb-row tile g (128 rows)
                items = []

                def rt_item(m, g=g):
                    if ("RT", g) not in p5_state:
                        p5_state[("RT", g)] = mp.tile([128, 8 * GC], bf16, tag="RTg",
                                                      name=f"RTg{g}", bufs=2)
                    RTg = p5_state[("RT", g)]
                    pr = mpp.tile([128, GC], fp32, tag="pr", name=f"pr{g}_{m}", bufs=1)
                    for k in range(8):
                        nc.tensor.matmul(
                            pr[:], w1t_sb[:, H * k + 128 * m:H * k + 128 * (m + 1)],
                            H1T3[:, 8 * g + 1:8 * (g + 1) + 1, 16 * k:16 * (k + 1)],
                            start=(k == 0), stop=(k == 7))
                    nc.scalar.activation(
                        RTg[:, GC * m:GC * (m + 1)], pr[:],
                        mybir.ActivationFunctionType.Relu, bias=b1_sb[:, m:m + 1], scale=1.0)

                def out_item(m, g=g):
                    if ("OUT", g) not in p5_state:
                        p5_state[("OUT", g)] = mp.tile([128, 4 * GC], bf16, tag="OUTg",
                                                       name=f"OUTg{g}", bufs=2)
                    RTg = p5_state[("RT", g)]
                    OUTg = p5_state[("OUT", g)]
                    po = mpp.tile([128, GC], fp32, tag="po", name=f"po{g}_{m}", bufs=1)
                    for k in range(8):
                        nc.tensor.matmul(
                            po[:], w2t_sb[:, E * k + 128 * m:E * k + 128 * (m + 1)],
                            RTg[:, GC * k:GC * (k + 1)],
                            start=(k == 0), stop=(k == 7))
                    nc.scalar.activation(
                        OUTg[:, GC * m:GC * (m + 1)], po[:],
                        mybir.ActivationFunctionType.Identity, bias=b2_sb[:, m:m + 1], scale=1.0)

                def lg_item(nchs, g=g):
                    if ("L", g) not in p5_state:
                        p5_state[("L", g)] = mp.tile([128, VS], fp32, tag="lsb",
                                                     name=f"lsb{g}", bufs=2)
                    OUTg = p5_state[("OUT", g)]
                    lsb = p5_state[("L", g)]
                    pl = mpp.tile([128, 500], fp32, tag="pl", name=f"pl{g}_{nchs}", bufs=2)
                    for e in range(4):
                        nc.tensor.matmul(
                            pl[:], OUTg[:, GC * e:GC * e + 128],
                            embts_sb[:, VS * e + nchs:VS * e + nchs + 500],
                            start=(e == 0), stop=(not USE_BIAS and e == 3))
                    if USE_BIAS:
                        nc.tensor.matmul(
                            pl[:], ones_sb[0:1, 0:128], bgen_sb[0:1, nchs:nchs + 500],
                            start=False, stop=True)
                    nc.vector.tensor_copy(lsb[:, nchs:nchs + 500], pl[:])

                def dma_item(g=g):
                    lsb = p5_state[("L", g)]
                    nc.gpsimd.dma_start(out[128 * g:128 * (g + 1), 0:VS // 2],
                                        lsb[:, 0:VS // 2])
                    nc.gpsimd.dma_start(out[128 * g:128 * (g + 1), VS // 2:VS],
                                        lsb[:, VS // 2:VS])

                for m in range(8):
                    items.append(lambda m=m: rt_item(m))
                for m in range(4):
                    items.append(lambda m=m: out_item(m))
                for nchs in range(0, VS, 500):
                    items.append(lambda n=nchs: lg_item(n))
                items.append(dma_item)
                return items

            p5_queue = []
            p5_next_g = 0

            # prefetch x-gates for the first two steps
            for u in (1, 2):
                nc.scalar.dma_start(
                    gis_ring[:, G3 * ((u - 1) % 4):G3 * ((u - 1) % 4) + G3],
                    GI_d[16 * (u - 1):16 * u, :])

            for t in range(1, T + 3):
                cc_src = cc_srcs[t]
                if t + 2 <= T:
                    u = t + 2
                    nc.scalar.dma_start(
                        gis_ring[:, G3 * ((u - 1) % 4):G3 * ((u - 1) % 4) + G3],
                        GI_d[16 * (u - 1):16 * u, :])
                if t <= T:
                    # layer 0: h0_t  (critical chain)
                    slot = (t - 1) % 4
                    psg = psp.tile([16, G3], fp32, tag="psg", name=f"psg{t}", bufs=1)
                    for k in range(8):
                        nc.tensor.matmul(
                            psg[:],
                            ring0[:, 128 * slot + 16 * k:128 * slot + 16 * k + 16],
                            whh0_sb[:, G3 * k:G3 * (k + 1)],
                            start=(k == 0), stop=(not USE_BIAS and k == 7))
                    if USE_BIAS:
                        nc.tensor.matmul(psg[:], ones_sb[0:1, 0:16], bb0p_sb[0:1, :],
                                         start=False, stop=True)
                    gis = gis_ring[:, G3 * ((t - 1) % 4):G3 * ((t - 1) % 4) + G3]
                    gru_gates(psg[:, 0:256], psg[:, 256:384],
                              gis[:, 0:256], gis[:, 256:384],
                              h_own0[(t - 1) % 2], h_own0[t % 2], "L0", t)
                    tp0 = psp.tile([128, 16], fp32, tag="tp", name=f"tp0_{t}", bufs=1)
                    nc.tensor.transpose(tp0[:], h_own0[t % 2][:], ident[0:16, 0:16])
                    nc.vector.tensor_copy(cc_src[:, 0:16], tp0[:])

                ccin = dr.tile([128, 32], bf16, tag="ccin", name=f"ccin{t}", bufs=2)
                ccout = dr.tile([128 * N, 32], bf16, tag="ccout", name=f"ccout{t}",
                                bufs=2, addr_space="Shared")
                nc.sync.dma_start(ccin[:], cc_src[:])
                nc.gpsimd.collective_compute(
                    "AllGather", mybir.AluOpType.bypass,
                    replica_groups=[list(range(N))],
                    ins=[ccin[:]], outs=[ccout[:]])
                if t <= T:
                    rsl = ring0[:, 128 * (t % 4):128 * (t % 4) + 128].rearrange("p (k c) -> p k c", k=8)
                    csl = ccout[:, 0:16].rearrange("(k p) c -> p k c", p=128)
                    nc.sync.dma_start(rsl[:, 0:4, :], csl[:, 0:4, :])
                    nc.sync.dma_start(rsl[:, 4:8, :], csl[:, 4:8, :])
                if t >= 3:
                    nc.scalar.dma_start(
                        H1T[:, 128 * (t - 2):128 * (t - 1)].rearrange("p (k c) -> p k c", k=8),
                        ccout[:, 16:32].rearrange("(k p) c -> p k c", p=128))

                if 2 <= t <= T + 1:
                    # layer 1: h1_{t-1} (rides AG t+1; PE does this during AG flight)
                    psA = psp.tile([16, G3], fp32, tag="psA", name=f"psA{t}", bufs=1)
                    for k in range(8):
                        nc.tensor.matmul(
                            psA[:],
                            H1T[:, 128 * (t - 2) + 16 * k:128 * (t - 2) + 16 * k + 16],
                            whh1_sb[:, G3 * k:G3 * (k + 1)],
                            start=(k == 0), stop=(not USE_BIAS and k == 7))
                    if USE_BIAS:
                        nc.tensor.matmul(psA[:], ones_sb[0:1, 0:16], bb1p_sb[0:1, :],
                                         start=False, stop=True)
                    psB = psp.tile([16, G3], fp32, tag="psB", name=f"psB{t}", bufs=1)
                    rslot = (t - 1) % 4
                    for k in range(8):
                        nc.tensor.matmul(
                            psB[:],
                            ring0[:, 128 * rslot + 16 * k:128 * rslot + 16 * k + 16],
                            wih1_sb[:, G3 * k:G3 * (k + 1)],
                            start=(k == 0), stop=(not USE_BIAS and k == 7))
                    if USE_BIAS:
                        nc.tensor.matmul(psB[:], ones_sb[0:1, 0:16], ba1_sb[0:1, :],
                                         start=False, stop=True)
                    sB = st.tile([16, G3], fp32, tag="sB", name=f"sB{t}", bufs=2)
                    nc.vector.tensor_copy(sB[:], psB[:])
                    gru_gates(psA[:, 0:256], psA[:, 256:384],
                              sB[:, 0:256], sB[:, 256:384],
                              h_own1[(t - 2) % 2], h_own1[(t - 1) % 2], "L1", t)
                    tp1 = psp.tile([128, 16], fp32, tag="tp", name=f"tp1_{t}", bufs=1)
                    nc.tensor.transpose(tp1[:], h_own1[(t - 1) % 2][:], ident[0:16, 0:16])
                    nc.vector.tensor_copy(cc_srcs[t + 1][:, 16:32], tp1[:])

                while p5_next_g < NG and t >= 8 * p5_next_g + 10:
                    p5_queue.extend(p5_items(p5_next_g))
                    p5_next_g += 1
                for _ in range(4):
                    if p5_queue:
                        p5_queue.pop(0)()

            while p5_next_g < NG:
                p5_queue.extend(p5_items(p5_next_g))
                p5_next_g += 1
            for it in p5_queue:
                it()
            p5_queue = []

            mpp_cm.__exit__(None, None, None)
            mp_cm.__exit__(None, None, None)
            psp_cm.__exit__(None, None, None)

    nc.finalize()
    return nc


def _prep_inputs(hidden, trg, embedding, w_ih0, w_hh0, b_ih0, b_hh0,
                 w_ih1, w_hh1, b_ih1, b_hh1, w1, b1, w2, b2, b_gen):
    bf = ml_dtypes.bfloat16
    T = STEPS
    f32 = np.float32
    hidden = np.asarray(hidden, f32)
    trg = np.asarray(trg)
    embedding = np.asarray(embedding, f32)
    in_maps = []
    # (t,b) index order
    idx_full = np.asarray(trg.T[:T], np.int32).reshape(-1)          # (T*B,)
    idx_tiles = idx_full.reshape(-1, 128).astype(np.int32)          # (NT,128)

    def gslice(wT, r):
        # wT (K, 3H) -> (K, 384) slice of each gate for core r
        cols = np.concatenate([np.arange(HS) + g * H + r * HS for g in range(3)])
        return np.ascontiguousarray(wT[:, cols])

    for r in range(N):
        sl = slice(r * HS, (r + 1) * HS)
        ba0_ = (b_ih0 + b_hh0).astype(f32)
        ba0v = np.concatenate([ba0_[0 * H + r * HS:0 * H + (r + 1) * HS],
                               ba0_[1 * H + r * HS:1 * H + (r + 1) * HS],
                               np.asarray(b_ih0, f32)[2 * H + r * HS:2 * H + (r + 1) * HS]])
        bb0v = np.concatenate([np.zeros(2 * HS, f32),
                               np.asarray(b_hh0, f32)[2 * H + r * HS:2 * H + (r + 1) * HS]])
        ba1_ = (b_ih1 + b_hh1).astype(f32)
        ba1v = np.concatenate([ba1_[0 * H + r * HS:0 * H + (r + 1) * HS],
                               ba1_[1 * H + r * HS:1 * H + (r + 1) * HS],
                               np.asarray(b_ih1, f32)[2 * H + r * HS:2 * H + (r + 1) * HS]])
        bb1v = np.concatenate([np.zeros(2 * HS, f32),
                               np.asarray(b_hh1, f32)[2 * H + r * HS:2 * H + (r + 1) * HS]])
        in_maps.append({
            "emb": embedding,
            "idx": idx_tiles,
            "h0f": hidden[0].T.astype(bf),
            "h1f": hidden[1].T.astype(bf),
            "h0o": np.ascontiguousarray(hidden[0][:, sl]).astype(f32),
            "h1o": np.ascontiguousarray(hidden[1][:, sl]).astype(f32),
            "wih0": gslice(np.asarray(w_ih0, f32).T, r).astype(bf),
            "whh0": gslice(np.asarray(w_hh0, f32).T, r).astype(bf),
            "wih1": gslice(np.asarray(w_ih1, f32).T, r).astype(bf),
            "whh1": gslice(np.asarray(w_hh1, f32).T, r).astype(bf),
            "ba0": ba0v.reshape(1, -1).astype(bf),
            "bb0p": bb0v.reshape(1, -1).astype(bf),
            "ba1": ba1v.reshape(1, -1).astype(bf),
            "bb1p": bb1v.reshape(1, -1).astype(bf),
            "w1t": np.asarray(w1, f32).T.astype(bf),
            "b1c": np.asarray(b1, f32).reshape(8, 128).T.astype(f32),
            "w2t": np.asarray(w2, f32).T.astype(bf),
            "b2c": np.asarray(b2, f32).reshape(4, 128).T.astype(f32),
            "embts": np.ascontiguousarray(embedding.T[:, r * VS:(r + 1) * VS]).astype(bf),
            "bgen": np.asarray(b_gen, f32)[r * VS:(r + 1) * VS].reshape(1, -1).astype(bf),
        })
    return in_maps


def kernel(**inputs):
    from concourse.bass_utils import run_bass_kernel_spmd
    zb = not any(np.asarray(inputs[k]).any() for k in
                 ("b_ih0", "b_hh0", "b_ih1", "b_hh1", "b1", "b2", "b_gen"))
    key = ("nc", zb)
    if key not in _cache:
        _cache[key] = _build(USE_BIAS=not zb)
    nc = _cache[key]
    in_maps = _prep_inputs(**inputs)
    res = run_bass_kernel_spmd(nc, in_maps, core_ids=list(range(N)))
    T = STEPS
    outf = np.empty((B, T, V), np.float32)
    for r in range(N):
        lr = res.results[r]["out"].reshape(T, B, VS)
        outf[:, :, r * VS:(r + 1) * VS] = lr.transpose(1, 0, 2)
    return outf
